# revision 7
# baseline (speedup 1.0000x reference)
"""EpisodicMemory distributed Trainium2 kernel (8 NeuronCores).

Memory bank sharded across 8 cores (8192 rows each). Exact-integer bf16
BitNet matmuls, hi/lo-split fp32-quality score and attn@v matmuls, flash
attention over 1024-row s-blocks, distributed softmax via AllReduce
(max, then sum of [o | sigma_p]), replicated consolidation MLP.
"""
import sys
sys.path.insert(0, "/opt/trn_rl_repo")
import numpy as np

from concourse import bass, bacc, tile, mybir, bass_utils, bass_isa

dt = mybir.dt
AL = mybir.AluOpType
AF = mybir.ActivationFunctionType
RO = bass_isa.ReduceOp

NC = 8
B, D, M = 2048, 512, 65536
S = M // NC               # 8192 memory rows per core
J1 = 2 * D
SB = 1024                 # flash s-block rows
NSB = S // SB             # 8 blocks
NST = SB // 128           # 8 s-tiles per block
NSC = SB // 512           # 2 score chunks per block
MAGIC = 12582912.0
T23 = float(np.float32(2.0 / 3.0))

_cache = {}


def build(flags):
    nc = bacc.Bacc("TRN2", target_bir_lowering=False, debug=False, num_devices=NC)

    ins = {}
    def dram_in(name, shape):
        ins[name] = nc.dram_tensor(name, list(shape), dt.float32, kind="ExternalInput")
        return ins[name]

    x_d = dram_in("x", [B, D])
    mem_d = dram_in("memory", [S, D])
    w = {}
    for p in ("q", "k", "v"):
        for nm in ("w1", "b1", "g", "be", "w2", "b2"):
            w[f"{p}_{nm}"] = dram_in(f"{p}_{nm}", flags["shapes"][f"{p}_{nm}"])
    for nm in ("c_w1", "c_b1", "c_g1", "c_be1", "c_w2", "c_b2", "c_g2", "c_be2"):
        w[nm] = dram_in(nm, flags["shapes"][nm])
    out_d = nc.dram_tensor("out", [B, D], dt.float32, kind="ExternalOutput")

    inv = float(1.0 / (np.sqrt(D) * flags["temperature"]))
    replica = [list(range(NC))]

    with tile.TileContext(nc) as tc:
        with tc.tile_pool(name="dram", bufs=1, space="DRAM") as dram, \
             tc.tile_pool(name="wp", bufs=1) as wp, \
             tc.tile_pool(name="pp", bufs=1) as pp, \
             tc.tile_pool(name="psum", bufs=4, space="PSUM") as ps, \
             tc.tile_pool(name="wk1", bufs=1) as wk1, \
             tc.tile_pool(name="wk2", bufs=2) as wk2, \
             tc.tile_pool(name="small", bufs=1) as sm:

            qtmp = wk1.tile([128, 1024], dt.float32, tag="qtmp")

            def split3(src_f32, hi_bf, lo_bf, tmp_f32):
                nc.vector.tensor_copy(hi_bf, src_f32)
                nc.vector.tensor_copy(tmp_f32, hi_bf)
                nc.vector.tensor_tensor(out=lo_bf, in0=src_f32, in1=tmp_f32, op=AL.subtract)

            # ---------- weight prep ----------
            def prep_weight(wt, jdim, ddim, name):
                npt = jdim // 128
                ndt = ddim // 128
                nat = wk1.tile([128, 1024], dt.float32, tag="wnat")
                colsum = sm.tile([128, 8], dt.float32, tag="wcs")
                for t in range(npt):
                    nc.sync.dma_start(out=nat[:, 0:ddim], in_=wt[t * 128:(t + 1) * 128, :])
                    nc.vector.tensor_reduce(colsum[:, t:t + 1], nat[:, 0:ddim],
                                            axis=mybir.AxisListType.X, op=AL.add,
                                            apply_absolute_value=True)
                tot = sm.tile([128, 1], dt.float32, tag="wtot")
                nc.vector.tensor_reduce(tot[:], colsum[:, 0:npt], axis=mybir.AxisListType.X,
                                        op=AL.add)
                tot_r = sm.tile([128, 1], dt.float32, tag="wtotr")
                nc.gpsimd.partition_all_reduce(tot_r[:], tot[:], channels=128, reduce_op=RO.add)
                scale = wp.tile([128, 1], dt.float32, tag=f"wsc_{name}")
                nc.vector.tensor_scalar(out=scale[:], in0=tot_r[:],
                                        scalar1=1.0 / (jdim * ddim), scalar2=1e-5,
                                        op0=AL.mult, op1=AL.max)
                nc.vector.tensor_scalar(out=scale[:], in0=scale[:], scalar1=1000.0,
                                        scalar2=None, op0=AL.min)
                rs = sm.tile([128, 1], dt.float32, tag="wrs")
                nc.vector.reciprocal(rs[:], scale[:])
                tern = wk1.tile([128, 1024], dt.bfloat16, tag="wtern")
                gt = qtmp
                tT = wp.tile([128, ndt, jdim], dt.bfloat16, tag=f"wT_{name}")
                for t in range(npt):
                    nc.sync.dma_start(out=nat[:, 0:ddim], in_=wt[t * 128:(t + 1) * 128, :])
                    nc.vector.tensor_scalar(out=nat[:, 0:ddim], in0=nat[:, 0:ddim],
                                            scalar1=rs[:], scalar2=None, op0=AL.mult)
                    nc.vector.tensor_scalar(out=gt[:, 0:ddim], in0=nat[:, 0:ddim], scalar1=T23,
                                            scalar2=None, op0=AL.is_gt)
                    nc.vector.tensor_scalar(out=nat[:, 0:ddim], in0=nat[:, 0:ddim],
                                            scalar1=-T23, scalar2=None, op0=AL.is_lt)
                    nc.vector.tensor_tensor(out=tern[:, 0:ddim], in0=gt[:, 0:ddim],
                                            in1=nat[:, 0:ddim], op=AL.subtract)
                    nc.sync.dma_start_transpose(tT[:, :, t * 128:(t + 1) * 128],
                                                tern[:, 0:ddim])
                return tT, scale

            TW, SW = {}, {}
            for p in ("q", "k", "v"):
                TW[f"{p}1"], SW[f"{p}1"] = prep_weight(w[f"{p}_w1"], D, D, f"{p}1")
                TW[f"{p}2"], SW[f"{p}2"] = prep_weight(w[f"{p}_w2"], D, D, f"{p}2")
            TW["c1"], SW["c1"] = prep_weight(w["c_w1"], J1, D, "c1")
            TW["c2"], SW["c2"] = prep_weight(w["c_w2"], D, J1, "c2")

            def bcast_row(vec_d, n, name):
                t = wp.tile([128, n], dt.float32, tag=f"row_{name}")
                nc.sync.dma_start(out=t[0:1, :], in_=vec_d[:].rearrange("a -> 1 a"))
                nc.gpsimd.partition_broadcast(t[:], t[0:1, :])
                return t

            rows = {}
            def need_row(key, flagkey, n):
                if not flags[flagkey]:
                    rows[key] = bcast_row(w[key], n, key)
            for p in ("q", "k", "v"):
                need_row(f"{p}_b1", f"{p}_b1_zero", D)
                need_row(f"{p}_g", f"{p}_g_one", D)
                need_row(f"{p}_be", f"{p}_be_zero", D)
            need_row("v_b2", "v_b2_zero", D)
            need_row("q_b2", "q_b2_zero", D)   # applied per-partition below instead
            need_row("c_b1", "c_b1_zero", J1)
            need_row("c_g1", "c_g1_one", J1)
            need_row("c_be1", "c_be1_zero", J1)
            need_row("c_b2", "c_b2_zero", D)
            need_row("c_g2", "c_g2_one", D)
            need_row("c_be2", "c_be2_zero", D)
            pbias = {}
            for p in ("q", "k"):
                if not flags[f"{p}_b2_zero"]:
                    t = wp.tile([128, 4], dt.float32, tag=f"pb_{p}")
                    nc.sync.dma_start(out=t[:], in_=w[f"{p}_b2"][:].rearrange("(t p) -> p t", p=128))
                    pbias[p] = t

            # ---------- helpers ----------
            def newton_rsqrt(r, t, tmp):
                for _ in range(2):
                    nc.vector.tensor_tensor(out=tmp, in0=r, in1=r, op=AL.mult)
                    nc.vector.tensor_tensor(out=tmp, in0=tmp, in1=t, op=AL.mult)
                    nc.vector.tensor_scalar(out=tmp, in0=tmp, scalar1=-0.5, scalar2=1.5,
                                            op0=AL.mult, op1=AL.add)
                    nc.vector.tensor_tensor(out=r, in0=r, in1=tmp, op=AL.mult)

            def rsqrt_of(var_col, name):
                rst = sm.tile([128, 1], dt.float32, tag=f"rst_{name}")
                nc.scalar.sqrt(rst[:], var_col)
                rin = sm.tile([128, 1], dt.float32, tag=f"rin_{name}")
                nc.vector.reciprocal(rin[:], rst[:])
                ntmp = sm.tile([128, 1], dt.float32, tag=f"ntmp_{name}")
                newton_rsqrt(rin[:], var_col, ntmp[:])
                return rin

            def layer_norm_gelu(ap, nfree, name, gkey, bekey, nstats=1):
                bst = sm.tile([128, 6 * nstats], dt.float32, tag=f"bst_{name}")
                for i in range(nstats):
                    nc.vector.bn_stats(bst[:, 6 * i:6 * i + 6], ap[:, 512 * i:512 * (i + 1)]
                                       if nstats > 1 else ap)
                bag = sm.tile([128, 2], dt.float32, tag=f"bag_{name}")
                nc.vector.bn_aggr(bag[:], bst[:])
                var = sm.tile([128, 1], dt.float32, tag=f"var_{name}")
                nc.vector.tensor_scalar(out=var[:], in0=bag[:, 1:2], scalar1=1e-5,
                                        scalar2=None, op0=AL.add)
                rin = rsqrt_of(var[:], name)
                nc.vector.tensor_scalar(out=ap, in0=ap, scalar1=bag[:, 0:1], scalar2=rin[:],
                                        op0=AL.subtract, op1=AL.mult)
                if gkey in rows:
                    nc.vector.tensor_tensor(out=ap, in0=ap, in1=rows[gkey][:, 0:nfree], op=AL.mult)
                if bekey in rows:
                    nc.vector.tensor_tensor(out=ap, in0=ap, in1=rows[bekey][:, 0:nfree], op=AL.add)
                nc.scalar.activation(ap, ap, AF.Gelu)

            def quant_consts(mx_ap, neg_mn_ap, name):
                rngv = sm.tile([1, 1], dt.float32, tag=f"qcr_{name}")
                nc.vector.tensor_tensor(out=rngv[:], in0=mx_ap, in1=neg_mn_ap, op=AL.add)
                sc = sm.tile([1, 1], dt.float32, tag=f"qcs_{name}")
                nc.vector.tensor_scalar(out=sc[:], in0=rngv[:], scalar1=1.0 / 255.0,
                                        scalar2=1e-8, op0=AL.mult, op1=AL.max)
                nc.vector.tensor_scalar(out=sc[:], in0=sc[:], scalar1=1000.0, scalar2=None,
                                        op0=AL.min)
                rs = sm.tile([1, 1], dt.float32, tag=f"qcrs_{name}")
                nc.vector.reciprocal(rs[:], sc[:])
                zp = sm.tile([1, 1], dt.float32, tag=f"qcz_{name}")
                nc.vector.tensor_tensor(out=zp[:], in0=neg_mn_ap, in1=rs[:], op=AL.mult)
                nc.vector.tensor_scalar(out=zp[:], in0=zp[:], scalar1=MAGIC, scalar2=MAGIC,
                                        op0=AL.add, op1=AL.subtract)
                nc.vector.tensor_scalar(out=zp[:], in0=zp[:], scalar1=0.0, scalar2=255.0,
                                        op0=AL.max, op1=AL.min)
                rs_b = wp.tile([128, 1], dt.float32, tag=f"qb_rs_{name}")
                zp_b = wp.tile([128, 1], dt.float32, tag=f"qb_zp_{name}")
                sc_b = wp.tile([128, 1], dt.float32, tag=f"qb_sc_{name}")
                nc.gpsimd.partition_broadcast(rs_b[:], rs[:])
                nc.gpsimd.partition_broadcast(zp_b[:], zp[:])
                nc.gpsimd.partition_broadcast(sc_b[:], sc[:])
                return rs_b, zp_b, sc_b

            def quantize_tile(out_bf, in_f32, rs_b, zp_b, n):
                tmp = qtmp[:, 0:n]
                nc.scalar.activation(tmp, in_f32, AF.Identity, bias=zp_b[:], scale=rs_b[:])
                nc.vector.tensor_scalar(out=tmp, in0=tmp, scalar1=MAGIC, scalar2=MAGIC,
                                        op0=AL.add, op1=AL.subtract)
                nc.vector.tensor_scalar(out=tmp, in0=tmp, scalar1=0.0, scalar2=255.0,
                                        op0=AL.max, op1=AL.min)
                nc.vector.tensor_scalar(out=out_bf, in0=tmp, scalar1=zp_b[:], scalar2=None,
                                        op0=AL.subtract)

            tmpc1 = sm.tile([128, 1], dt.float32, tag="tmpc1")

            def freeminmax(dst2, src):
                nc.vector.tensor_reduce(dst2[:, 0:1], src, axis=mybir.AxisListType.X, op=AL.max)
                nc.vector.tensor_reduce(tmpc1[:], src, axis=mybir.AxisListType.X, op=AL.min)
                nc.vector.tensor_scalar(out=dst2[:, 1:2], in0=tmpc1[:], scalar1=-1.0,
                                        scalar2=None, op0=AL.mult)

            def minmax_finish(acc, ncols, name):
                mm = sm.tile([128, 2], dt.float32, tag=f"mmf_{name}")
                nc.vector.tensor_reduce(mm[:, 0:1], acc[:, 0:ncols:2],
                                        axis=mybir.AxisListType.X, op=AL.max)
                nc.vector.tensor_reduce(mm[:, 1:2], acc[:, 1:ncols:2],
                                        axis=mybir.AxisListType.X, op=AL.max)
                mmr = sm.tile([128, 2], dt.float32, tag=f"mmr_{name}")
                nc.gpsimd.partition_all_reduce(mmr[:], mm[:], channels=128, reduce_op=RO.max)
                return mmr

            stage = wk2.tile([128, 4, D], dt.float32, tag="stage")      # shared f32 staging
            A_st = wk2.tile([128, 4, D], dt.bfloat16, tag="A_st")       # quantized chunk
            A_stT = wk2.tile([128, 4, D], dt.bfloat16, tag="A_stT")     # transposed chunk
            AT_slot = pp.tile([128, 8, B], dt.bfloat16, tag="AT_slot")  # A_xT/A_hqT/A_rT/A_h1T
            f512 = wk1.tile([128, 512], dt.float32, tag="f512")
            f512b = wk1.tile([128, 512], dt.float32, tag="f512b")

            hq_dram = dram.tile([B, D], dt.float32)
            hk_dram = dram.tile([S, D], dt.float32)
            hv_dram = dram.tile([S, D], dt.float32)

            # ================= PHASE 1: q path =================
            xmm = sm.tile([128, 8], dt.float32, tag="xmm")
            for ch in range(4):
                for t in range(4):
                    r0 = ch * 512 + t * 128
                    nc.sync.dma_start(out=stage[:, t, :], in_=x_d[r0:r0 + 128, :])
                mml = sm.tile([128, 8], dt.float32, tag="mml")
                for t in range(4):
                    freeminmax(mml[:, 2 * t:2 * t + 2], stage[:, t, :])
                nc.vector.tensor_reduce(xmm[:, 2 * ch:2 * ch + 1], mml[:, 0::2],
                                        axis=mybir.AxisListType.X, op=AL.max)
                nc.vector.tensor_reduce(xmm[:, 2 * ch + 1:2 * ch + 2], mml[:, 1::2],
                                        axis=mybir.AxisListType.X, op=AL.max)
            xr = minmax_finish(xmm, 8, "x")
            rs_x, zp_x, sc_x = quant_consts(xr[0:1, 0:1], xr[0:1, 1:2], "x")
            A_xT = AT_slot[:, 0:4, :]
            for ch in range(4):
                for t in range(4):
                    r0 = ch * 512 + t * 128
                    nc.sync.dma_start(out=stage[:, t, :], in_=x_d[r0:r0 + 128, :])
                    quantize_tile(A_st[:, t, :], stage[:, t, :], rs_x, zp_x, D)
                    nc.sync.dma_start_transpose(A_xT[:, :, r0:r0 + 128], A_st[:, t, :])
            c_q1 = sm.tile([128, 1], dt.float32, tag="c_q1")
            nc.vector.tensor_tensor(out=c_q1[:], in0=sc_x[:], in1=SW["q1"][:], op=AL.mult)
            hqmm = sm.tile([128, 32], dt.float32, tag="hqmm")
            for bt in range(16):
                acc = ps.tile([128, D], dt.float32, tag="pA")
                for dtl in range(4):
                    nc.tensor.matmul(acc[:], A_xT[:, dtl, bt * 128:(bt + 1) * 128],
                                     TW["q1"][:, dtl, :], start=(dtl == 0), stop=(dtl == 3))
                hq_t = f512[:]
                nc.vector.tensor_scalar(out=hq_t, in0=acc[:], scalar1=c_q1[:], scalar2=None,
                                        op0=AL.mult)
                if "q_b1" in rows:
                    nc.vector.tensor_tensor(out=hq_t, in0=hq_t, in1=rows["q_b1"][:], op=AL.add)
                layer_norm_gelu(hq_t, D, "hq", "q_g", "q_be")
                freeminmax(hqmm[:, 2 * bt:2 * bt + 2], hq_t)
                nc.sync.dma_start(out=hq_dram[bt * 128:(bt + 1) * 128, :], in_=hq_t)
            hqr = minmax_finish(hqmm, 32, "hq")
            rs_hq, zp_hq, sc_hq = quant_consts(hqr[0:1, 0:1], hqr[0:1, 1:2], "hq")
            A_hqT = AT_slot[:, 0:4, :]
            for ch in range(4):
                for t in range(4):
                    r0 = ch * 512 + t * 128
                    nc.sync.dma_start(out=stage[:, t, :], in_=hq_dram[r0:r0 + 128, :])
                    quantize_tile(A_st[:, t, :], stage[:, t, :], rs_hq, zp_hq, D)
                    nc.sync.dma_start_transpose(A_hqT[:, :, r0:r0 + 128], A_st[:, t, :])
            c_q2 = sm.tile([128, 1], dt.float32, tag="c_q2")
            nc.vector.tensor_tensor(out=c_q2[:], in0=sc_hq[:], in1=SW["q2"][:], op=AL.mult)
            q_hi = pp.tile([128, 4, B], dt.bfloat16, tag="q_hi")
            q_lo = pp.tile([128, 4, B], dt.bfloat16, tag="q_lo")
            for dtl in range(4):
                for bc in range(4):
                    acc = ps.tile([128, 512], dt.float32, tag="pA")
                    for jt in range(4):
                        nc.tensor.matmul(acc[:], TW["q2"][:, jt, dtl * 128:(dtl + 1) * 128],
                                         A_hqT[:, jt, bc * 512:(bc + 1) * 512],
                                         start=(jt == 0), stop=(jt == 3))
                    nc.vector.tensor_scalar(out=f512[:], in0=acc[:], scalar1=c_q2[:],
                                            scalar2=None, op0=AL.mult)
                    if "q" in pbias:
                        nc.vector.tensor_scalar(out=f512[:], in0=f512[:],
                                                scalar1=pbias["q"][:, dtl:dtl + 1],
                                                scalar2=None, op0=AL.add)
                    split3(f512[:], q_hi[:, dtl, bc * 512:(bc + 1) * 512],
                           q_lo[:, dtl, bc * 512:(bc + 1) * 512], f512b[:])

            # ============ PHASE 2: memory minmax + h pass ============
            mmm = sm.tile([128, 32], dt.float32, tag="memmm")
            for ch in range(16):
                for t in range(4):
                    r0 = ch * 512 + t * 128
                    nc.sync.dma_start(out=stage[:, t, :], in_=mem_d[r0:r0 + 128, :])
                mml = sm.tile([128, 8], dt.float32, tag="mml")
                for t in range(4):
                    freeminmax(mml[:, 2 * t:2 * t + 2], stage[:, t, :])
                nc.vector.tensor_reduce(mmm[:, 2 * ch:2 * ch + 1], mml[:, 0::2],
                                        axis=mybir.AxisListType.X, op=AL.max)
                nc.vector.tensor_reduce(mmm[:, 2 * ch + 1:2 * ch + 2], mml[:, 1::2],
                                        axis=mybir.AxisListType.X, op=AL.max)
            memr = minmax_finish(mmm, 32, "mem")
            cc_in1 = dram.tile([1, 2], dt.float32)
            cc_out1 = dram.tile([1, 2], dt.float32, addr_space="Shared")
            nc.sync.dma_start(out=cc_in1[:], in_=memr[0:1, :])
            nc.gpsimd.collective_compute("AllReduce", AL.max, ins=[cc_in1[:]],
                                         outs=[cc_out1[:]], replica_groups=replica)
            memg = sm.tile([1, 2], dt.float32, tag="memg")
            nc.sync.dma_start(out=memg[:], in_=cc_out1[:])
            rs_m, zp_m, sc_m = quant_consts(memg[0:1, 0:1], memg[0:1, 1:2], "mem")
            c_k1 = sm.tile([128, 1], dt.float32, tag="c_k1")
            c_v1 = sm.tile([128, 1], dt.float32, tag="c_v1")
            nc.vector.tensor_tensor(out=c_k1[:], in0=sc_m[:], in1=SW["k1"][:], op=AL.mult)
            nc.vector.tensor_tensor(out=c_v1[:], in0=sc_m[:], in1=SW["v1"][:], op=AL.mult)

            hmm = sm.tile([128, 64], dt.float32, tag="hmm")
            for ch in range(16):
                for t in range(4):
                    r0 = ch * 512 + t * 128
                    nc.sync.dma_start(out=stage[:, t, :], in_=mem_d[r0:r0 + 128, :])
                    quantize_tile(A_st[:, t, :], stage[:, t, :], rs_m, zp_m, D)
                    nc.sync.dma_start_transpose(A_stT[:, :, t * 128:(t + 1) * 128],
                                                A_st[:, t, :])
                hmml = sm.tile([128, 16], dt.float32, tag="hmml")
                for ti, (pfx, cc, hd) in enumerate((("k", c_k1, hk_dram), ("v", c_v1, hv_dram))):
                    for t in range(4):
                        s0 = ch * 512 + t * 128
                        acc = ps.tile([128, D], dt.float32, tag="pA")
                        for dtl in range(4):
                            nc.tensor.matmul(acc[:], A_stT[:, dtl, t * 128:(t + 1) * 128],
                                             TW[f"{pfx}1"][:, dtl, :],
                                             start=(dtl == 0), stop=(dtl == 3))
                        hs = f512[:]
                        nc.vector.tensor_scalar(out=hs, in0=acc[:], scalar1=cc[:],
                                                scalar2=None, op0=AL.mult)
                        if f"{pfx}_b1" in rows:
                            nc.vector.tensor_tensor(out=hs, in0=hs, in1=rows[f"{pfx}_b1"][:],
                                                    op=AL.add)
                        layer_norm_gelu(hs, D, "h", f"{pfx}_g", f"{pfx}_be")
                        freeminmax(hmml[:, 8 * ti + 2 * t:8 * ti + 2 * t + 2], hs)
                        nc.sync.dma_start(out=hd[s0:s0 + 128, :], in_=hs)
                for ci, sl in ((0, slice(0, 8, 2)), (1, slice(1, 8, 2)),
                               (2, slice(8, 16, 2)), (3, slice(9, 16, 2))):
                    nc.vector.tensor_reduce(hmm[:, 4 * ch + ci:4 * ch + ci + 1], hmml[:, sl],
                                            axis=mybir.AxisListType.X, op=AL.max)
            hall = sm.tile([128, 4], dt.float32, tag="hall")
            for ci in range(4):
                nc.vector.tensor_reduce(hall[:, ci:ci + 1], hmm[:, ci::4],
                                        axis=mybir.AxisListType.X, op=AL.max)
            hallr = sm.tile([128, 4], dt.float32, tag="hallr")
            nc.gpsimd.partition_all_reduce(hallr[:], hall[:], channels=128, reduce_op=RO.max)
            cc_in2 = dram.tile([1, 4], dt.float32)
            cc_out2 = dram.tile([1, 4], dt.float32, addr_space="Shared")
            nc.sync.dma_start(out=cc_in2[:], in_=hallr[0:1, :])
            nc.gpsimd.collective_compute("AllReduce", AL.max, ins=[cc_in2[:]],
                                         outs=[cc_out2[:]], replica_groups=replica)
            hg = sm.tile([1, 4], dt.float32, tag="hg")
            nc.sync.dma_start(out=hg[:], in_=cc_out2[:])
            rs_hk, zp_hk, sc_hk = quant_consts(hg[0:1, 0:1], hg[0:1, 1:2], "hk")
            rs_hv, zp_hv, sc_hv = quant_consts(hg[0:1, 2:3], hg[0:1, 3:4], "hv")
            c_k2 = sm.tile([128, 1], dt.float32, tag="c_k2")
            c_v2 = sm.tile([128, 1], dt.float32, tag="c_v2")
            nc.vector.tensor_tensor(out=c_k2[:], in0=sc_hk[:], in1=SW["k2"][:], op=AL.mult)
            nc.vector.tensor_tensor(out=c_v2[:], in0=sc_hv[:], in1=SW["v2"][:], op=AL.mult)

            # ============ PHASE 3+4: per s-block k/v build + flash ============
            m_blk = pp.tile([128, 16, NSB], dt.float32, tag="m_blk")
            s_blk = pp.tile([128, 16, NSB], dt.float32, tag="s_blk")
            o_dram = dram.tile([NSB, B, D], dt.float32)

            k_hi = pp.tile([128, 4, SB], dt.bfloat16, tag="k_hi")
            k_lo = pp.tile([128, 4, SB], dt.bfloat16, tag="k_lo")
            v_hi = pp.tile([128, NST, D], dt.bfloat16, tag="v_hi")
            v_lo = pp.tile([128, NST, D], dt.bfloat16, tag="v_lo")
            sc_buf = wk1.tile([128, SB], dt.float32, tag="sc_buf")
            p_f32 = wk1.tile([128, SB], dt.float32, tag="p_f32")
            p_hi = wk1.tile([128, SB], dt.bfloat16, tag="wtern")
            p_lo = wk1.tile([128, SB], dt.bfloat16, tag="p_lo")
            pT_hi = wk1.tile([128, NST, 128], dt.bfloat16, tag="pT_hi")
            pT_lo = wk1.tile([128, NST, 128], dt.bfloat16, tag="pT_lo")

            for blk in range(NSB):
                base = blk * SB
                for ch in range(NSC):
                    for pfx in ("k", "v"):
                        hd = hk_dram if pfx == "k" else hv_dram
                        rs_h = rs_hk if pfx == "k" else rs_hv
                        zp_h = zp_hk if pfx == "k" else zp_hv
                        for t in range(4):
                            s0 = base + ch * 512 + t * 128
                            nc.sync.dma_start(out=stage[:, t, :], in_=hd[s0:s0 + 128, :])
                            quantize_tile(A_st[:, t, :], stage[:, t, :], rs_h, zp_h, D)
                            nc.sync.dma_start_transpose(A_stT[:, :, t * 128:(t + 1) * 128],
                                                        A_st[:, t, :])
                        if pfx == "k":
                            for dtl in range(4):
                                acc = ps.tile([128, 512], dt.float32, tag="pA")
                                for jt in range(4):
                                    nc.tensor.matmul(acc[:],
                                                     TW["k2"][:, jt, dtl * 128:(dtl + 1) * 128],
                                                     A_stT[:, jt, :], start=(jt == 0),
                                                     stop=(jt == 3))
                                nc.vector.tensor_scalar(out=f512[:], in0=acc[:], scalar1=c_k2[:],
                                                        scalar2=None, op0=AL.mult)
                                if "k" in pbias:
                                    nc.vector.tensor_scalar(out=f512[:], in0=f512[:],
                                                            scalar1=pbias["k"][:, dtl:dtl + 1],
                                                            scalar2=None, op0=AL.add)
                                split3(f512[:], k_hi[:, dtl, ch * 512:(ch + 1) * 512],
                                       k_lo[:, dtl, ch * 512:(ch + 1) * 512], f512b[:])
                        else:
                            for t in range(4):
                                st = ch * 4 + t
                                acc = ps.tile([128, 512], dt.float32, tag="pA")
                                for jt in range(4):
                                    nc.tensor.matmul(acc[:], A_stT[:, jt, t * 128:(t + 1) * 128],
                                                     TW["v2"][:, jt, :], start=(jt == 0),
                                                     stop=(jt == 3))
                                nc.vector.tensor_scalar(out=f512[:], in0=acc[:], scalar1=c_v2[:],
                                                        scalar2=None, op0=AL.mult)
                                if "v_b2" in rows:
                                    nc.vector.tensor_tensor(out=f512[:], in0=f512[:],
                                                            in1=rows["v_b2"][:], op=AL.add)
                                split3(f512[:], v_hi[:, st, :], v_lo[:, st, :], f512b[:])

                for bt in range(16):
                    mxc = sm.tile([128, NSC], dt.float32, tag="mxc")
                    for sci in range(NSC):
                        acc = ps.tile([128, 512], dt.float32, tag="pA")
                        mm_i = 0
                        for (qa, ka) in ((q_hi, k_hi), (q_hi, k_lo), (q_lo, k_hi)):
                            for dtl in range(4):
                                nc.tensor.matmul(acc[:], qa[:, dtl, bt * 128:(bt + 1) * 128],
                                                 ka[:, dtl, sci * 512:(sci + 1) * 512],
                                                 start=(mm_i == 0), stop=(mm_i == 11))
                                mm_i += 1
                        nc.scalar.copy(sc_buf[:, sci * 512:(sci + 1) * 512], acc[:])
                        nc.vector.tensor_reduce(mxc[:, sci:sci + 1], acc[:],
                                                axis=mybir.AxisListType.X, op=AL.max)
                    nc.vector.tensor_reduce(m_blk[:, bt, blk:blk + 1], mxc[:],
                                            axis=mybir.AxisListType.X, op=AL.max)
                    nmb = sm.tile([128, 1], dt.float32, tag="nmb")
                    nc.vector.tensor_scalar(out=nmb[:], in0=m_blk[:, bt, blk:blk + 1],
                                            scalar1=-inv, scalar2=None, op0=AL.mult)
                    nc.scalar.activation(p_f32[:], sc_buf[:], AF.Exp, bias=nmb[:], scale=inv,
                                         accum_out=s_blk[:, bt, blk:blk + 1])
                    nc.scalar.activation(p_hi[:], sc_buf[:], AF.Exp, bias=nmb[:], scale=inv)
                    nc.vector.tensor_tensor(out=p_lo[:], in0=p_f32[:], in1=p_hi[:],
                                            op=AL.subtract)
                    nc.sync.dma_start_transpose(pT_hi[:], p_hi[:])
                    nc.sync.dma_start_transpose(pT_lo[:], p_lo[:])
                    acc = ps.tile([128, D], dt.float32, tag="pA")
                    mm_i = 0
                    nmm = 3 * NST
                    for (pa, va) in ((pT_hi, v_hi), (pT_hi, v_lo), (pT_lo, v_hi)):
                        for st in range(NST):
                            nc.tensor.matmul(acc[:], pa[:, st, :], va[:, st, :],
                                             start=(mm_i == 0), stop=(mm_i == nmm - 1))
                            mm_i += 1
                    nc.vector.tensor_copy(f512[:], acc[:])
                    nc.sync.dma_start(out=o_dram[blk, bt * 128:(bt + 1) * 128, :], in_=f512[:])

            # ============ PHASE 5: softmax merge (local blocks + cross-core) ============
            m_loc = pp.tile([128, 16], dt.float32, tag="m_loc")
            for bt in range(16):
                nc.vector.tensor_reduce(m_loc[:, bt:bt + 1], m_blk[:, bt, :],
                                        axis=mybir.AxisListType.X, op=AL.max)
            m_bounce = dram.tile([B, 1], dt.float32)
            m_out = dram.tile([B, 1], dt.float32, addr_space="Shared")
            for bt in range(16):
                nc.sync.dma_start(out=m_bounce[bt * 128:(bt + 1) * 128, :], in_=m_loc[:, bt:bt + 1])
            nc.gpsimd.collective_compute("AllReduce", AL.max, ins=[m_bounce[:]],
                                         outs=[m_out[:]], replica_groups=replica)
            so_in = dram.tile([B, D + 1], dt.float32)
            so_out = dram.tile([B, D + 1], dt.float32, addr_space="Shared")
            for bt in range(16):
                m_g = sm.tile([128, 1], dt.float32, tag="m_g")
                nc.sync.dma_start(out=m_g[:], in_=m_out[bt * 128:(bt + 1) * 128, :])
                fdel = sm.tile([128, NSB], dt.float32, tag="fdel")
                nc.vector.tensor_scalar(out=fdel[:], in0=m_blk[:, bt, :], scalar1=m_g[:],
                                        scalar2=inv, op0=AL.subtract, op1=AL.mult)
                fexp = sm.tile([128, NSB], dt.float32, tag="fexp")
                nc.scalar.activation(fexp[:], fdel[:], AF.Exp)
                sscaled = sm.tile([128, NSB], dt.float32, tag="sscaled")
                nc.vector.tensor_tensor(out=sscaled[:], in0=s_blk[:, bt, :], in1=fexp[:],
                                        op=AL.mult)
                ssum = sm.tile([128, 1], dt.float32, tag="ssum")
                nc.vector.tensor_reduce(ssum[:], sscaled[:], axis=mybir.AxisListType.X, op=AL.add)
                oacc = f512[:]
                nc.vector.memset(oacc, 0.0)
                for blk in range(NSB):
                    nc.sync.dma_start(out=f512b[:], in_=o_dram[blk, bt * 128:(bt + 1) * 128, :])
                    nc.vector.scalar_tensor_tensor(out=oacc, in0=f512b[:],
                                                   scalar=fexp[:, blk:blk + 1], in1=oacc,
                                                   op0=AL.mult, op1=AL.add)
                nc.sync.dma_start(out=so_in[bt * 128:(bt + 1) * 128, 0:D], in_=oacc)
                nc.sync.dma_start(out=so_in[bt * 128:(bt + 1) * 128, D:D + 1], in_=ssum[:])
            nc.gpsimd.collective_compute("AllReduce", AL.add, ins=[so_in[:]],
                                         outs=[so_out[:]], replica_groups=replica)

            retr_dram = dram.tile([B, D], dt.float32)
            rmm = sm.tile([128, 32], dt.float32, tag="rmm")
            for bt in range(16):
                nc.sync.dma_start(out=f512[:], in_=so_out[bt * 128:(bt + 1) * 128, 0:D])
                sg = sm.tile([128, 1], dt.float32, tag="sg")
                nc.sync.dma_start(out=sg[:], in_=so_out[bt * 128:(bt + 1) * 128, D:D + 1])
                rg = sm.tile([128, 1], dt.float32, tag="rg")
                nc.vector.reciprocal(rg[:], sg[:])
                nc.vector.tensor_scalar(out=f512[:], in0=f512[:], scalar1=rg[:],
                                        scalar2=None, op0=AL.mult)
                freeminmax(rmm[:, 2 * bt:2 * bt + 2], f512[:])
                nc.sync.dma_start(out=retr_dram[bt * 128:(bt + 1) * 128, :], in_=f512[:])

            # ============ PHASE 6: consolidation (replicated) ============
            rr = minmax_finish(rmm, 32, "retr")
            rs_r, zp_r, sc_r = quant_consts(rr[0:1, 0:1], rr[0:1, 1:2], "retr")
            A_rT = AT_slot[:, 0:4, :]
            for ch in range(4):
                for t in range(4):
                    r0 = ch * 512 + t * 128
                    nc.sync.dma_start(out=stage[:, t, :], in_=retr_dram[r0:r0 + 128, :])
                    quantize_tile(A_st[:, t, :], stage[:, t, :], rs_r, zp_r, D)
                    nc.sync.dma_start_transpose(A_rT[:, :, r0:r0 + 128], A_st[:, t, :])
            c_c1 = sm.tile([128, 1], dt.float32, tag="c_c1")
            nc.vector.tensor_tensor(out=c_c1[:], in0=sc_r[:], in1=SW["c1"][:], op=AL.mult)
            h1_dram = dram.tile([B, J1], dt.float32)
            h1mm = sm.tile([128, 32], dt.float32, tag="h1mm")
            h1t = qtmp
            for bt in range(16):
                for jc in range(2):
                    acc = ps.tile([128, 512], dt.float32, tag="pA")
                    for dtl in range(4):
                        nc.tensor.matmul(acc[:], A_rT[:, dtl, bt * 128:(bt + 1) * 128],
                                         TW["c1"][:, dtl, jc * 512:(jc + 1) * 512],
                                         start=(dtl == 0), stop=(dtl == 3))
                    nc.vector.tensor_scalar(out=h1t[:, jc * 512:(jc + 1) * 512], in0=acc[:],
                                            scalar1=c_c1[:], scalar2=None, op0=AL.mult)
                if "c_b1" in rows:
                    nc.vector.tensor_tensor(out=h1t[:], in0=h1t[:], in1=rows["c_b1"][:], op=AL.add)
                layer_norm_gelu(h1t[:], J1, "c1", "c_g1", "c_be1", nstats=2)
                freeminmax(h1mm[:, 2 * bt:2 * bt + 2], h1t[:])
                nc.sync.dma_start(out=h1_dram[bt * 128:(bt + 1) * 128, :], in_=h1t[:])
            h1r = minmax_finish(h1mm, 32, "h1")
            rs_h1, zp_h1, sc_h1 = quant_consts(h1r[0:1, 0:1], h1r[0:1, 1:2], "h1")
            A_h1T = AT_slot[:]
            for bt in range(16):
                r0 = bt * 128
                nc.sync.dma_start(out=qtmp[:], in_=h1_dram[r0:r0 + 128, :])
                A1 = wk1.tile([128, J1], dt.bfloat16, tag="wtern")
                quantize_tile(A1[:], qtmp[:], rs_h1, zp_h1, J1)
                nc.sync.dma_start_transpose(A_h1T[:, :, r0:r0 + 128], A1[:])
            c_c2 = sm.tile([128, 1], dt.float32, tag="c_c2")
            nc.vector.tensor_tensor(out=c_c2[:], in0=sc_h1[:], in1=SW["c2"][:], op=AL.mult)
            for bt in range(16):
                acc = ps.tile([128, D], dt.float32, tag="pA")
                for jt in range(8):
                    nc.tensor.matmul(acc[:], A_h1T[:, jt, bt * 128:(bt + 1) * 128],
                                     TW["c2"][:, jt, :], start=(jt == 0), stop=(jt == 7))
                nc.vector.tensor_scalar(out=f512[:], in0=acc[:], scalar1=c_c2[:],
                                        scalar2=None, op0=AL.mult)
                if "c_b2" in rows:
                    nc.vector.tensor_tensor(out=f512[:], in0=f512[:], in1=rows["c_b2"][:],
                                            op=AL.add)
                layer_norm_gelu2 = True
                bst = sm.tile([128, 6], dt.float32, tag="bst_o")
                bag = sm.tile([128, 2], dt.float32, tag="bag_o")
                nc.vector.bn_stats(bst[:], f512[:])
                nc.vector.bn_aggr(bag[:], bst[:])
                var = sm.tile([128, 1], dt.float32, tag="var_o")
                nc.vector.tensor_scalar(out=var[:], in0=bag[:, 1:2], scalar1=1e-5,
                                        scalar2=None, op0=AL.add)
                rin = rsqrt_of(var[:], "o")
                nc.vector.tensor_scalar(out=f512[:], in0=f512[:], scalar1=bag[:, 0:1],
                                        scalar2=rin[:], op0=AL.subtract, op1=AL.mult)
                if "c_g2" in rows:
                    nc.vector.tensor_tensor(out=f512[:], in0=f512[:], in1=rows["c_g2"][:],
                                            op=AL.mult)
                if "c_be2" in rows:
                    nc.vector.tensor_tensor(out=f512[:], in0=f512[:], in1=rows["c_be2"][:],
                                            op=AL.add)
                nc.sync.dma_start(out=out_d[bt * 128:(bt + 1) * 128, :], in_=f512[:])

    nc.compile()
    return nc


def kernel(**inputs):
    inputs = {k: np.asarray(v) for k, v in inputs.items()}
    flags = {"temperature": float(inputs["temperature"])}
    flags["shapes"] = {k: list(np.shape(v)) for k, v in inputs.items()}
    for p in ("q", "k", "v"):
        flags[f"{p}_b1_zero"] = bool(np.all(inputs[f"{p}_b1"] == 0))
        flags[f"{p}_b2_zero"] = bool(np.all(inputs[f"{p}_b2"] == 0))
        flags[f"{p}_be_zero"] = bool(np.all(inputs[f"{p}_be"] == 0))
        flags[f"{p}_g_one"] = bool(np.all(inputs[f"{p}_g"] == 1))
    for nm in ("c_b1", "c_be1", "c_b2", "c_be2"):
        flags[f"{nm}_zero"] = bool(np.all(inputs[nm] == 0))
    for nm in ("c_g1", "c_g2"):
        flags[f"{nm}_one"] = bool(np.all(inputs[nm] == 1))

    key = (flags["temperature"],) + tuple(sorted((k, v) for k, v in flags.items()
                                                 if isinstance(v, bool)))
    if key not in _cache:
        _cache[key] = build(flags)
    nc = _cache[key]

    in_maps = []
    for c in range(NC):
        im = {k: np.ascontiguousarray(v, dtype=np.float32) for k, v in inputs.items()
              if k not in ("memory", "temperature")}
        im["memory"] = np.ascontiguousarray(inputs["memory"][c * S:(c + 1) * S, :],
                                            dtype=np.float32)
        in_maps.append(im)
    res = bass_utils.run_bass_kernel_spmd(nc, in_maps, core_ids=list(range(NC)))
    return np.asarray(res.results[0]["out"], dtype=np.float32)
